# revision 1
# baseline (speedup 1.0000x reference)
"""Trainium2 Bass kernel: Classical STDP weight update.

Math (matches the jax reference with TAU_PLUS == TAU_MINUS, so both
eligibility traces are identical and eff = (A_PLUS - A_MINUS) * trace):

    trace[t, b, p] = sum_{s < t} decay^(t-s) * pre[s, b, p]
    dW[p, q] = (A+ - A-)/(B*T) * sum_{t, b} trace[t,b,p] * post[t,b,q]

The trace is a lower-triangular Toeplitz matmul per batch element:
trace_b = L @ pre_b with L[t, s] = decay^(t-s) (t > s).  So per batch
element b the whole computation is two chained TensorE matmuls:

    stage 1: trace_b [T, 256]  = (c*L^T).T @ pre_b        (c folded into L)
    stage 2: dW_h   [128, 256] += trace_b[:, h*128:...].T @ post_b

Sharding: data-parallel over the batch (512 -> 64 per core on 8 cores);
each core emits a partial dW [256, 256]; the 8 partials are summed on
the host (the /(B*T) mean commutes with the reduction).

Spikes are 0/1 so bf16 inputs are lossless and halve HBM traffic.
PSUM accumulates in fp32 throughout.

Sync-wait budget note: this toolchain's walrus rejects any compute/DMA
instruction carrying more than ONE semaphore wait, and the final
all-engine drain gets one wait per DMA lane + engine used.  Hence:
  * pre, post and L^T are packed into a single DRAM tensor so each DMA
    group is ONE dma_start (one lane, one sem tick).  After stage 1
    waits on that lane, the PE's vector clock covers the post data too,
    so stage-2 matmuls only ever wait on the DVE trace copy.
  * dedicated SBUF buffers (no recycling) keep input DMAs at zero waits.
  * the trace-PSUM pool has 5 bufs so a stage-1 matmul's slot-reuse WAR
    is covered by a DVE tick the PE has already observed.
"""

import numpy as np
import ml_dtypes

# Problem constants (hardcoded per the harness contract).
B, T, N_PRE, N_POST = 512, 100, 256, 256
N_CORES = 8
B_SHARD = B // N_CORES  # 64
A_PLUS, A_MINUS = 0.005, 0.00525
TAU_PLUS = 20.0
DT_ = 1.0

GROUP = 8    # batch elements per input DMA group (8 groups)
CHUNK = 8    # batch elements per compute chunk (trace tile / stage-2 unit)
SUB = 2      # batch elements per stage-1 matmul (2*256 = 512 cols = 1 bank)

N_GROUPS = B_SHARD // GROUP
N_ROWS = 2 * B_SHARD  # pre/post rows in the packed fp8 tensor

_NC_CACHE = {}


def _lt_matrix() -> np.ndarray:
    """c * L^T as f32: LT[s, t] = (A+ - A-) * decay^(t-s) for t > s else 0."""
    decay = np.exp(np.float64(-DT_ / TAU_PLUS))
    idx = np.arange(T)
    diff = idx[None, :] - idx[:, None]  # t - s
    lt = np.where(diff > 0, (A_PLUS - A_MINUS) * decay ** diff, 0.0)
    return lt.astype(np.float32)


def _build(repeat=1, split=True):
    """Build the per-core Bass program (shard of 64 batch elements).

    repeat>1 unrolls the whole body N times inside one NEFF (bench only:
    wall-clock slope over N isolates device time from dispatch overhead).
    """
    import concourse.bass as bass
    import concourse.tile as tile
    from concourse import mybir

    f32 = mybir.dt.float32
    bf16 = mybir.dt.bfloat16
    f8 = mybir.dt.float8e4
    ds = bass.ds

    nc = bass.Bass()
    # Packed fp8 input, t-major ([T, rows, P]) so each DMA partition row is
    # one long contiguous read (spikes are 0/1 so fp8e4 is lossless and
    # halves HBM traffic again).
    data_d = nc.declare_dram_parameter("data", [T, N_ROWS, N_PRE], f8, isOutput=False)
    lt_d = nc.declare_dram_parameter("lt", [T, T], bf16, isOutput=False)
    dw_d = nc.declare_dram_parameter("dw", [N_PRE, N_POST], f32, isOutput=True)

    chunks_per_group = GROUP // CHUNK
    n_chunks = B_SHARD // CHUNK
    n_sub = CHUNK // SUB

    with tile.TileContext(nc) as tc:
        with (
            tc.tile_pool(name="const", bufs=1) as cpool,
            tc.tile_pool(name="io", bufs=1) as io_pool,
            tc.tile_pool(name="tr", bufs=n_chunks) as tr_pool,
            tc.tile_pool(name="psum", bufs=5, space="PSUM") as ps_pool,
            tc.tile_pool(name="acc", bufs=1, space="PSUM") as acc_pool,
        ):
            # dW accumulators: one PSUM bank per 128-row half of dW.
            dw_ps = [
                acc_pool.tile([128, N_POST], f32, tag=f"dw{h}", name=f"dw_ps{h}")
                for h in range(2)
            ]

            def stage2(trace_t, post_rows, first, last):
                for bi in range(CHUNK):
                    for h in range(2):
                        nc.tensor.matmul(
                            dw_ps[h][:],
                            trace_t[:, bi, ds(h * 128, 128)],
                            post_rows[:, bi, :],
                            start=first and bi == 0,
                            stop=last and bi == CHUNK - 1,
                            skip_group_check=True,
                        )

            def emit_body():
                lt_tile = cpool.tile([T, T], bf16, tag="lt", name="lt_tile")
                nc.sync.dma_start(lt_tile[:], lt_d[:])
                lt_t = lt_tile[:]
                pending = []  # (trace_t, post_rows)
                emitted = 0
                chunk_i = 0
                for g in range(N_GROUPS):
                    lo = 2 * GROUP * g
                    grp_t = io_pool.tile([T, 2 * GROUP, N_PRE], f8, tag=f"grp{g}",
                                         name=f"grp_t{g}")
                    if g == 0:
                        # Split group 0 so stage 1 starts after just the pre
                        # half of the first transfer has landed.
                        nc.sync.dma_start(
                            grp_t[:, 0:GROUP, :], data_d[:, lo : lo + GROUP, :])
                        nc.sync.dma_start(
                            grp_t[:, GROUP : 2 * GROUP, :],
                            data_d[:, lo + GROUP : lo + 2 * GROUP, :])
                    else:
                        nc.sync.dma_start(
                            grp_t[:], data_d[:, lo : lo + 2 * GROUP, :])
                    off = 0
                    for cc in range(chunks_per_group):
                        pbase = cc * CHUNK
                        trace_t = tr_pool.tile([T, CHUNK, N_PRE], bf16, tag="trace")
                        for j in range(n_sub):
                            tr_ps = ps_pool.tile([T, SUB, N_PRE], f32, tag="trps")
                            nc.tensor.matmul(
                                tr_ps[:],
                                lt_t,
                                grp_t[:, ds(off + pbase + j * SUB, SUB), :],
                                start=True,
                                stop=True,
                                skip_group_check=True,
                            )
                            # Alternate copy engine per sub-block: copies of
                            # one chunk run on both engines concurrently, so
                            # they finish before the next chunk's stage-1
                            # matmuls do (PE never stalls on the trace).
                            if j % 2 == 0:
                                nc.vector.tensor_copy(
                                    trace_t[:, ds(j * SUB, SUB), :], tr_ps[:]
                                )
                            else:
                                nc.scalar.copy(
                                    trace_t[:, ds(j * SUB, SUB), :], tr_ps[:]
                                )
                        chunk_i += 1
                        post_rows = grp_t[:, ds(off + GROUP + pbase, CHUNK), :]
                        pending.append((trace_t, post_rows))
                        # Skew stage 2 one chunk behind so PE never stalls on
                        # the copy of the chunk it just produced.
                        if len(pending) >= 2:
                            args = pending.pop(0)
                            stage2(*args, first=(emitted == 0), last=False)
                            emitted += 1
                args = pending.pop(0)
                stage2(*args, first=(emitted == 0), last=True)

                # Bounce PSUM -> SBUF, then one store via SWDGE (gpsimd): its
                # queue has no prior traffic, so the DMA needs one sync wait.
                out_sb = cpool.tile([128, 2, N_POST], f32, tag="osb",
                                    name="out_sb")
                for h in range(2):
                    nc.vector.tensor_copy(out_sb[:, h, :], dw_ps[h][:])
                nc.gpsimd.dma_start(
                    dw_d[:].rearrange("(h p) q -> p h q", h=2), out_sb[:]
                )

            for _rep in range(repeat):
                emit_body()

    if split:
        _split_multiwaits(nc)
    return nc


def _split_multiwaits(nc):
    """Walrus on this toolchain allows one sync wait per instruction; hoist
    extra waits onto single-wait NOPs preceding the instruction (sequential
    sem-ge waits are equivalent to a combined wait)."""
    from concourse import mybir

    for fn in nc.m.functions:
        for bb in fn.blocks:
            out = []
            changed = False
            for inst in bb.instructions:
                si = inst.sync_info
                waits = list(si.on_wait) if (si is not None and si.on_wait) else []
                if len(waits) > 1:
                    changed = True
                    for w in waits[:-1]:
                        out.append(mybir.InstNoOp(
                            name=nc.get_next_instruction_name(),
                            ins=[], outs=[],
                            sync_info=mybir.SyncInfo(on_wait=[w], on_update=[]),
                            bass_nofuse=True,
                            engine=inst.engine,
                        ))
                    si.on_wait = waits[-1:]
                out.append(inst)
            if changed:
                bb.instructions = out


def _get_nc():
    if "nc" not in _NC_CACHE:
        _NC_CACHE["nc"] = _build()
    return _NC_CACHE["nc"]


def _pack_core(pre_b, post_b):
    """Build the packed t-major [T, N_ROWS, N_PRE] fp8 tensor for one core."""
    out = np.zeros((T, N_ROWS, N_PRE), dtype=ml_dtypes.float8_e4m3)
    for g in range(N_GROUPS):
        lo = 2 * GROUP * g
        sl = slice(g * GROUP, (g + 1) * GROUP)
        out[:, lo : lo + GROUP] = pre_b[sl].transpose(1, 0, 2)
        out[:, lo + GROUP : lo + 2 * GROUP] = post_b[sl].transpose(1, 0, 2)
    return out


def _make_in_maps(pre_spikes, post_spikes):
    pre = np.asarray(pre_spikes, dtype=np.float32).astype(ml_dtypes.float8_e4m3)
    post = np.asarray(post_spikes, dtype=np.float32).astype(ml_dtypes.float8_e4m3)
    lt = _lt_matrix().astype(ml_dtypes.bfloat16)
    return [
        {
            "data": _pack_core(
                pre[i * B_SHARD : (i + 1) * B_SHARD],
                post[i * B_SHARD : (i + 1) * B_SHARD],
            ),
            "lt": np.ascontiguousarray(lt),
        }
        for i in range(N_CORES)
    ]


def kernel(pre_spikes, post_spikes, weights=None, **unused):
    from concourse.bass_utils import run_bass_kernel_spmd

    nc = _get_nc()
    in_maps = _make_in_maps(pre_spikes, post_spikes)
    res = run_bass_kernel_spmd(nc, in_maps, core_ids=list(range(N_CORES)))
    partial = np.stack([r["dw"] for r in res.results])  # [8, 256, 256] f32
    dw = partial.sum(axis=0) / np.float32(B * T)
    return dw.astype(np.float32)

